# revision 11
# baseline (speedup 1.0000x reference)
"""Trainium2 Bass kernel for nn_CachedMoEExperts (MoE routing, E=16, top-4).

Strategy (expert-parallel, host-side dispatch):
  - Host computes the (tiny) router: softmax -> top-4 -> renormalize.
  - Tokens are gathered per expert on the host; experts are paired
    big-with-small and assigned 2 per NeuronCore (16 experts / 8 cores).
  - Each core runs the expert FFN y = gate * (w2 @ silu(w1 @ x_g^T)) for its
    two experts with fp32r matmuls (full-rate PE) on zero-padded token
    batches (slot capacities C0/C1, fixed at compile time).
  - Weights are host-packed into contiguous [128, kt, 128] panels so weight
    DMA runs at full bandwidth; x is passed pre-transposed so the kernel
    needs no on-chip transposes (weights are the stationary operand).
  - Host scatter-adds the per-expert outputs back into the [T, H] result.
"""

from contextlib import ExitStack

import numpy as np

import concourse.bacc as bacc
import concourse.bass as bass
import concourse.mybir as mybir
import concourse.tile as tile
from concourse.bass_utils import run_bass_kernel_spmd

F32 = mybir.dt.float32
F32R = mybir.dt.float32r
FP16 = mybir.dt.float16

# compute dtype for matmul operands: "f32r" (rel err ~2e-4) or
# "fp16" (halves weight/activation DMA, rel err ~1e-3)
COMPUTE_DT = "fp16"

NUM_EXPERTS = 16
TOP_K = 4
HIDDEN = 2048
INTER = 1408
TOKENS = 4096
N_CORES = 8

KT1 = HIDDEN // 128  # 16 contraction tiles for mm1
MT1 = INTER // 128   # 11 output-row tiles for mm1
KT2 = INTER // 128   # 11 contraction tiles for mm2
MT2 = HIDDEN // 128  # 16 output-row tiles for mm2

# Default slot capacities (tokens routed per expert; avg load is T*K/E=1024).
CAP0_DEFAULT = 1152  # the 8 most-loaded experts
CAP1_DEFAULT = 1024  # the 8 least-loaded experts

XG_KQ = 4  # xg sub-DMA granularity (kt tiles per transfer)

_PROGRAM_CACHE: dict = {}


def _ceil128(n: int) -> int:
    return max(128, (int(n) + 127) // 128 * 128)


def _cap64(n: int) -> int:
    # exact token capacity rounded to 64 (DMA alignment), at least 256
    return max(256, (int(n) + 63) // 64 * 64)


def _cap(n: int) -> int:
    # exact capacity, but at least 256 and even (keep chunk tails >=256 viable)
    return max(256, int(n) + (int(n) & 1))


def _plan_chunks(C: int):
    """Split the token capacity into moving-dim chunks <=512, all >=256 when
    possible (fp32r matmul runs at full rate only for free dim >=256)."""
    chunks = []
    off, rem = 0, C
    while rem > 0:
        if rem >= 768 or rem <= 512:
            sz = min(512, rem)
        else:  # 513..767: leave a 256 tail
            sz = rem - 256
        chunks.append((off, sz))
        off += sz
        rem -= sz
    return chunks


def _build_program(C0: int, C1: int, reps: int = 1, cdt_name: str | None = None,
                   unroll: int = 1):
    caps = (C0, C1)
    fp16 = (cdt_name or COMPUTE_DT) == "fp16"
    CDT = FP16 if fp16 else F32R
    ODT = FP16 if fp16 else F32  # y output dtype
    nc = bacc.Bacc("TRN2", debug=False, target_bir_lowering=False)

    xg_d = [
        nc.dram_tensor(f"xg{s}", (HIDDEN, caps[s]), CDT, kind="ExternalInput")
        for s in range(2)
    ]
    g_d = [
        nc.dram_tensor(f"g{s}", (128, caps[s]), F32, kind="ExternalInput")
        for s in range(2)
    ]
    y_d = [
        nc.dram_tensor(f"y{s}", (HIDDEN, caps[s]), ODT, kind="ExternalOutput")
        for s in range(2)
    ]
    w1_d = nc.dram_tensor(
        "w1p", (2, MT1, 128, KT1, 128), CDT, kind="ExternalInput"
    )
    w2_d = nc.dram_tensor(
        "w2p", (2, MT2, 128, KT2, 128), CDT, kind="ExternalInput"
    )

    slot_chunks = [_plan_chunks(caps[s]) for s in range(2)]

    with tile.TileContext(nc) as tc, ExitStack() as ctx:
        xgp = ctx.enter_context(tc.tile_pool(name="xg", bufs=1))
        wp = ctx.enter_context(tc.tile_pool(name="w", bufs=3))
        h1p = ctx.enter_context(tc.tile_pool(name="h1", bufs=1))
        gp = ctx.enter_context(tc.tile_pool(name="g", bufs=1))
        pp = ctx.enter_context(
            tc.tile_pool(name="psum", bufs=6, space=bass.MemorySpace.PSUM)
        )
        op = ctx.enter_context(tc.tile_pool(name="out", bufs=6))
        if reps > 1:
            ctx.enter_context(tc.For_i(0, reps, 1))

        NPRE = 3  # w1 tiles prefetched at iteration head

        for u in range(unroll):
          # Queue discipline (two HWDGE rings, each strictly FIFO):
          #   sync (SP) ring: w1 + xg -- everything the PE needs at the START
          #     of an iteration. Their buffer-free sems resolve mid-iteration,
          #     so the next iteration's loads stream while this one computes.
          #   scalar (ACT) ring: g, w2 + y stores -- paced by compute, safe to
          #     head-of-line block behind each other.
          # w1[0:NPRE] is prefetched before xg so the first mm1 group never
          # waits on a weight delivery at the loop back-edge.
          w1_tiles: dict = {}
          for m in range(NPRE):
              w1_tiles[m] = wp.tile([128, KT1, 128], CDT, tag="w1",
                                    name=f"w1_u{u}_s0_{m}")
              nc.sync.dma_start(w1_tiles[m][:], w1_d.ap()[0, m])

          xg_tiles: dict = {}
          g_ts: dict = {}
          for s in range(2):
            xg_view = xg_d[s].ap().rearrange("(kt p) n -> p kt n", p=128)
            for ci, (off, szn) in enumerate(slot_chunks[s]):
                xg_tiles[s, ci] = xgp.tile(
                    [128, KT1, szn], CDT, tag=f"xg{s}_{ci}",
                    name=f"xg_u{u}_s{s}_c{ci}"
                )
            for kq in range(0, KT1, XG_KQ):
                for ci, (off, szn) in enumerate(slot_chunks[s]):
                    nc.sync.dma_start(
                        xg_tiles[s, ci][:, kq : kq + XG_KQ, :],
                        xg_view[:, kq : kq + XG_KQ, off : off + szn],
                    )
            g_ts[s] = gp.tile([128, caps[s]], F32, tag=f"g{s}",
                              name=f"g_u{u}_s{s}")
            nc.scalar.dma_start(g_ts[s][:], g_d[s].ap()[:, :])

          for s in range(2):
            C = caps[s]
            chunks = slot_chunks[s]
            g_t = g_ts[s]

            h1_tiles = [
                h1p.tile([128, C], CDT, tag=f"h1_{s}_{m}",
                         name=f"h1_u{u}_s{s}_{m}")
                for m in range(MT1)
            ]

            # mm1 + silu: h1[i, t] = silu(sum_h w1[i, h] * x[t, h])
            # kt_outer: each weight tile feeds all chunks back-to-back
            for m in range(MT1):
                if s == 0 and m < NPRE:
                    wt = w1_tiles[m]
                else:
                    wt = wp.tile([128, KT1, 128], CDT, tag="w1",
                                 name=f"w1_u{u}_s{s}_{m}")
                    nc.sync.dma_start(wt[:], w1_d.ap()[s, m])
                pss = [
                    pp.tile([128, szn], F32, tag=f"ps{ci}", bufs=2,
                            name=f"ps1_u{u}_s{s}_{m}_{ci}")
                    for ci, (off, szn) in enumerate(chunks)
                ]
                for kt in range(KT1):
                    for ci, (off, szn) in enumerate(chunks):
                        nc.tensor.matmul(
                            pss[ci][:],
                            wt[:, kt, :],
                            xg_tiles[s, ci][:, kt, :],
                            start=(kt == 0),
                            stop=(kt == KT1 - 1),
                        )
                for ci, (off, szn) in enumerate(chunks):
                    nc.scalar.activation(
                        h1_tiles[m][:, off : off + szn],
                        pss[ci][:],
                        mybir.ActivationFunctionType.Silu,
                    )

            # mm2 + gate: y[hh, t] = g[t] * sum_i w2[hh, i] * h1[i, t]
            for m2 in range(MT2):
                wt2 = wp.tile([128, KT2, 128], CDT, tag="w2",
                              name=f"w2_u{u}_s{s}_{m2}")
                nc.scalar.dma_start(wt2[:], w2_d.ap()[s, m2])
                pss = [
                    pp.tile([128, szn], F32, tag=f"ps{ci}", bufs=2,
                            name=f"ps2_u{u}_s{s}_{m2}_{ci}")
                    for ci, (off, szn) in enumerate(chunks)
                ]
                for kt in range(KT2):
                    for ci, (off, szn) in enumerate(chunks):
                        nc.tensor.matmul(
                            pss[ci][:],
                            wt2[:, kt, :],
                            h1_tiles[kt][:, off : off + szn],
                            start=(kt == 0),
                            stop=(kt == KT2 - 1),
                        )
                for ci, (off, szn) in enumerate(chunks):
                    ot = op.tile([128, szn], ODT, tag="out",
                                 name=f"ot_u{u}_s{s}_{m2}_{ci}")
                    nc.vector.tensor_mul(ot[:], pss[ci][:],
                                         g_t[:, off : off + szn])
                    nc.scalar.dma_start(
                        y_d[s].ap()[m2 * 128 : (m2 + 1) * 128, off : off + szn],
                        ot[:],
                    )

    nc.compile()
    return nc


def _get_program(C0: int, C1: int):
    key = (C0, C1, COMPUTE_DT)
    if key not in _PROGRAM_CACHE:
        _PROGRAM_CACHE[key] = _build_program(C0, C1)
    return _PROGRAM_CACHE[key]


def build_for_bench(reps: int):
    """Rebuild the program of the last kernel() call with a reps-loop."""
    C0, C1 = kernel._bench_state["caps"]
    return _build_program(C0, C1, reps=reps)


def _route(router_logits: np.ndarray):
    """softmax -> top-4 (desc, ties by lower index) -> renormalize; matches
    jax.nn.softmax + jax.lax.top_k semantics in float32."""
    logits = router_logits.astype(np.float32, copy=False)
    m = logits.max(axis=-1, keepdims=True)
    e = np.exp(logits - m)
    probs = e / e.sum(axis=-1, keepdims=True)
    top_idx = np.argsort(-probs, axis=-1, kind="stable")[:, :TOP_K]
    top_vals = np.take_along_axis(probs, top_idx, axis=-1)
    top_vals = top_vals / top_vals.sum(axis=-1, keepdims=True)
    return top_idx.astype(np.int64), top_vals.astype(np.float32)


def _np_cdt():
    return np.float16 if COMPUTE_DT == "fp16" else np.float32


def _pack_w1(w1e: np.ndarray) -> np.ndarray:
    # [I, H] -> [MT1, 128, KT1, 128] with [m, p, kt, j] = w1e[m*128+j, kt*128+p]
    return np.ascontiguousarray(
        w1e.reshape(MT1, 128, KT1, 128).transpose(0, 3, 2, 1).astype(_np_cdt())
    )


def _pack_w2(w2e: np.ndarray) -> np.ndarray:
    # [H, I] -> [MT2, 128, KT2, 128] with [m, p, kt, j] = w2e[m*128+j, kt*128+p]
    return np.ascontiguousarray(
        w2e.reshape(MT2, 128, KT2, 128).transpose(0, 3, 2, 1).astype(_np_cdt())
    )


def _prepare(x, router_logits, w1, w2):
    x = np.ascontiguousarray(np.asarray(x, dtype=np.float32))
    router_logits = np.asarray(router_logits, dtype=np.float32)
    w1 = np.asarray(w1, dtype=np.float32)
    w2 = np.asarray(w2, dtype=np.float32)
    T = x.shape[0]

    top_idx, top_gates = _route(router_logits)

    flat_e = top_idx.ravel()
    flat_t = np.repeat(np.arange(T), TOP_K)
    flat_g = top_gates.ravel()
    order = np.argsort(flat_e, kind="stable")
    st, sg = flat_t[order], flat_g[order]
    counts = np.bincount(flat_e, minlength=NUM_EXPERTS)
    starts = np.concatenate([[0], np.cumsum(counts)])
    toks = [st[starts[e] : starts[e + 1]] for e in range(NUM_EXPERTS)]
    gs = [sg[starts[e] : starts[e + 1]] for e in range(NUM_EXPERTS)]

    # pair the most-loaded expert with the least-loaded, 2 experts per core
    rank = np.argsort(-counts, kind="stable")
    big = rank[:N_CORES]
    small = rank[N_CORES:][::-1]  # big[i] pairs with small[i]

    C0 = _cap64(counts[big].max())
    C1 = _cap64(counts[small].max())
    nc = _get_program(C0, C1)

    in_maps = []
    for c in range(N_CORES):
        pair = (int(big[c]), int(small[c]))
        caps = (C0, C1)
        im = {}
        for s, e in enumerate(pair):
            n = int(counts[e])
            xg = np.zeros((HIDDEN, caps[s]), _np_cdt())
            xg[:, :n] = x[toks[e]].T
            g = np.zeros((caps[s],), np.float32)
            g[:n] = gs[e]
            im[f"xg{s}"] = xg
            im[f"g{s}"] = np.broadcast_to(g, (128, caps[s])).copy()
        im["w1p"] = np.stack([_pack_w1(w1[e]) for e in pair])
        im["w2p"] = np.stack([_pack_w2(w2[e]) for e in pair])
        in_maps.append(im)

    meta = dict(T=T, counts=counts, toks=toks, big=big, small=small,
                caps=(C0, C1))
    return nc, in_maps, meta


def _combine(results, meta):
    out = np.zeros((meta["T"], HIDDEN), np.float32)
    for c in range(N_CORES):
        for s, e in enumerate((int(meta["big"][c]), int(meta["small"][c]))):
            n = int(meta["counts"][e])
            y = results[c][f"y{s}"]  # [HIDDEN, Cs], already gate-scaled
            out[meta["toks"][e]] += y[:, :n].T.astype(np.float32)
    return out


def kernel(x, router_logits, w1, w2):
    nc, in_maps, meta = _prepare(x, router_logits, w1, w2)
    res = run_bass_kernel_spmd(nc, in_maps, core_ids=list(range(N_CORES)))
    kernel._last_results = res
    kernel._bench_state = {"in_maps": in_maps, "caps": meta["caps"]}
    return _combine(res.results, meta)



# revision 16
# speedup vs baseline: 1.0078x; 1.0078x over previous
"""Trainium2 Bass kernel for nn_CachedMoEExperts (MoE routing, E=16, top-4).

Strategy (expert-parallel, host-side dispatch):
  - Host computes the (tiny) router: softmax -> top-4 -> renormalize.
  - Tokens are gathered per expert on the host; experts are paired
    big-with-small and assigned 2 per NeuronCore (16 experts / 8 cores).
  - Each core runs the expert FFN y = gate * (w2 @ silu(w1 @ x_g^T)) for its
    two experts with fp32r matmuls (full-rate PE) on zero-padded token
    batches (slot capacities C0/C1, fixed at compile time).
  - Weights are host-packed into contiguous [128, kt, 128] panels so weight
    DMA runs at full bandwidth; x is passed pre-transposed so the kernel
    needs no on-chip transposes (weights are the stationary operand).
  - Host scatter-adds the per-expert outputs back into the [T, H] result.
"""

from contextlib import ExitStack

import numpy as np

import concourse.bacc as bacc
import concourse.bass as bass
import concourse.mybir as mybir
import concourse.tile as tile
from concourse.bass_utils import run_bass_kernel_spmd

F32 = mybir.dt.float32
F32R = mybir.dt.float32r
FP16 = mybir.dt.float16

# compute dtype for matmul operands: "f32r" (rel err ~2e-4) or
# "fp16" (halves weight/activation DMA, rel err ~1e-3)
COMPUTE_DT = "fp16"

NUM_EXPERTS = 16
TOP_K = 4
HIDDEN = 2048
INTER = 1408
TOKENS = 4096
N_CORES = 8

KT1 = HIDDEN // 128  # 16 contraction tiles for mm1
MT1 = INTER // 128   # 11 output-row tiles for mm1
KT2 = INTER // 128   # 11 contraction tiles for mm2
MT2 = HIDDEN // 128  # 16 output-row tiles for mm2

# Default slot capacities (tokens routed per expert; avg load is T*K/E=1024).
CAP0_DEFAULT = 1152  # the 8 most-loaded experts
CAP1_DEFAULT = 1024  # the 8 least-loaded experts

XG_KQ = 8  # xg sub-DMA granularity (kt tiles per transfer)

_PROGRAM_CACHE: dict = {}


def _ceil128(n: int) -> int:
    return max(128, (int(n) + 127) // 128 * 128)


def _cap64(n: int) -> int:
    # exact token capacity rounded to 64 (DMA alignment), at least 256
    return max(256, (int(n) + 63) // 64 * 64)


def _cap(n: int) -> int:
    # exact capacity, but at least 256 and even (keep chunk tails >=256 viable)
    return max(256, int(n) + (int(n) & 1))


def _plan_chunks(C: int):
    """Split the token capacity into moving-dim chunks <=512, all >=256 when
    possible (fp32r matmul runs at full rate only for free dim >=256)."""
    chunks = []
    off, rem = 0, C
    while rem > 0:
        if rem >= 768 or rem <= 512:
            sz = min(512, rem)
        else:  # 513..767: leave a 256 tail
            sz = rem - 256
        chunks.append((off, sz))
        off += sz
        rem -= sz
    return chunks


def _build_program(C0: int, C1: int, reps: int = 1, cdt_name: str | None = None,
                   unroll: int = 1):
    caps = (C0, C1)
    fp16 = (cdt_name or COMPUTE_DT) == "fp16"
    CDT = FP16 if fp16 else F32R
    ODT = FP16 if fp16 else F32  # y output dtype
    nc = bacc.Bacc("TRN2", debug=False, target_bir_lowering=False)

    xg_d = [
        nc.dram_tensor(f"xg{s}", (HIDDEN, caps[s]), CDT, kind="ExternalInput")
        for s in range(2)
    ]
    g_d = [
        nc.dram_tensor(f"g{s}", (128, caps[s]), CDT, kind="ExternalInput")
        for s in range(2)
    ]
    y_d = [
        nc.dram_tensor(f"y{s}", (HIDDEN, caps[s]), ODT, kind="ExternalOutput")
        for s in range(2)
    ]
    w1_d = nc.dram_tensor(
        "w1p", (2, MT1, 128, KT1, 128), CDT, kind="ExternalInput"
    )
    w2_d = nc.dram_tensor(
        "w2p", (2, MT2, 128, KT2, 128), CDT, kind="ExternalInput"
    )

    slot_chunks = [_plan_chunks(caps[s]) for s in range(2)]

    with tile.TileContext(nc) as tc, ExitStack() as ctx:
        xgp = ctx.enter_context(tc.tile_pool(name="xg", bufs=1))
        wp = ctx.enter_context(tc.tile_pool(name="w", bufs=3))
        h1p = ctx.enter_context(tc.tile_pool(name="h1", bufs=1))
        gp = ctx.enter_context(tc.tile_pool(name="g", bufs=1))
        pp = ctx.enter_context(
            tc.tile_pool(name="psum", bufs=6, space=bass.MemorySpace.PSUM)
        )
        op = ctx.enter_context(tc.tile_pool(name="out", bufs=6))
        if reps > 1:
            ctx.enter_context(tc.For_i(0, reps, 1))

        NPRE = 3  # w1 tiles prefetched at iteration head

        for u in range(unroll):
          # Queue discipline (two HWDGE rings, each strictly FIFO):
          #   sync (SP) ring: w1 + xg -- everything the PE needs at the START
          #     of an iteration. Their buffer-free sems resolve mid-iteration,
          #     so the next iteration's loads stream while this one computes.
          #   scalar (ACT) ring: g, w2 + y stores -- paced by compute, safe to
          #     head-of-line block behind each other.
          # w1[0:NPRE] is prefetched before xg so the first mm1 group never
          # waits on a weight delivery at the loop back-edge.
          w1_tiles: dict = {}
          for m in range(NPRE):
              w1_tiles[m] = wp.tile([128, KT1, 128], CDT, tag="w1",
                                    name=f"w1_u{u}_s0_{m}")
              nc.sync.dma_start(w1_tiles[m][:], w1_d.ap()[0, m])

          xg_tiles: dict = {}
          g_ts: dict = {}
          for s in range(2):
            xg_view = xg_d[s].ap().rearrange("(kt p) n -> p kt n", p=128)
            for ci, (off, szn) in enumerate(slot_chunks[s]):
                xg_tiles[s, ci] = xgp.tile(
                    [128, KT1, szn], CDT, tag=f"xg{s}_{ci}",
                    name=f"xg_u{u}_s{s}_c{ci}"
                )
            for kq in range(0, KT1, XG_KQ):
                for ci, (off, szn) in enumerate(slot_chunks[s]):
                    nc.sync.dma_start(
                        xg_tiles[s, ci][:, kq : kq + XG_KQ, :],
                        xg_view[:, kq : kq + XG_KQ, off : off + szn],
                    )
            g_ts[s] = gp.tile([128, caps[s]], CDT, tag=f"g{s}",
                              name=f"g_u{u}_s{s}")
            nc.scalar.dma_start(g_ts[s][:], g_d[s].ap()[:, :])

          for s in range(2):
            C = caps[s]
            chunks = slot_chunks[s]
            g_t = g_ts[s]

            h1_tiles = [
                h1p.tile([128, C], CDT, tag=f"h1_{s}_{m}",
                         name=f"h1_u{u}_s{s}_{m}")
                for m in range(MT1)
            ]

            # mm1 + silu: h1[i, t] = silu(sum_h w1[i, h] * x[t, h])
            # kt_outer: each weight tile feeds all chunks back-to-back
            for m in range(MT1):
                if s == 0 and m < NPRE:
                    wt = w1_tiles[m]
                else:
                    wt = wp.tile([128, KT1, 128], CDT, tag="w1",
                                 name=f"w1_u{u}_s{s}_{m}")
                    nc.sync.dma_start(wt[:], w1_d.ap()[s, m])
                pss = [
                    pp.tile([128, szn], F32, tag=f"ps{ci}", bufs=2,
                            name=f"ps1_u{u}_s{s}_{m}_{ci}")
                    for ci, (off, szn) in enumerate(chunks)
                ]
                for kt in range(KT1):
                    for ci, (off, szn) in enumerate(chunks):
                        nc.tensor.matmul(
                            pss[ci][:],
                            wt[:, kt, :],
                            xg_tiles[s, ci][:, kt, :],
                            start=(kt == 0),
                            stop=(kt == KT1 - 1),
                        )
                for ci, (off, szn) in enumerate(chunks):
                    nc.scalar.activation(
                        h1_tiles[m][:, off : off + szn],
                        pss[ci][:],
                        mybir.ActivationFunctionType.Silu,
                    )

            # mm2 + gate: y[hh, t] = g[t] * sum_i w2[hh, i] * h1[i, t]
            for m2 in range(MT2):
                wt2 = wp.tile([128, KT2, 128], CDT, tag="w2",
                              name=f"w2_u{u}_s{s}_{m2}")
                nc.scalar.dma_start(wt2[:], w2_d.ap()[s, m2])
                pss = [
                    pp.tile([128, szn], F32, tag=f"ps{ci}", bufs=2,
                            name=f"ps2_u{u}_s{s}_{m2}_{ci}")
                    for ci, (off, szn) in enumerate(chunks)
                ]
                for kt in range(KT2):
                    for ci, (off, szn) in enumerate(chunks):
                        nc.tensor.matmul(
                            pss[ci][:],
                            wt2[:, kt, :],
                            h1_tiles[kt][:, off : off + szn],
                            start=(kt == 0),
                            stop=(kt == KT2 - 1),
                        )
                ot = op.tile([128, C], ODT, tag="out",
                             name=f"ot_u{u}_s{s}_{m2}")
                for ci, (off, szn) in enumerate(chunks):
                    nc.vector.tensor_mul(ot[:, off : off + szn], pss[ci][:],
                                         g_t[:, off : off + szn])
                nc.sync.dma_start(
                    y_d[s].ap()[m2 * 128 : (m2 + 1) * 128, :], ot[:]
                )

    nc.compile()
    return nc


def _get_program(C0: int, C1: int):
    key = (C0, C1, COMPUTE_DT)
    if key not in _PROGRAM_CACHE:
        _PROGRAM_CACHE[key] = _build_program(C0, C1)
    return _PROGRAM_CACHE[key]


def build_for_bench(reps: int):
    """Rebuild the program of the last kernel() call with a reps-loop."""
    C0, C1 = kernel._bench_state["caps"]
    return _build_program(C0, C1, reps=reps)


def _route(router_logits: np.ndarray):
    """softmax -> top-4 (desc, ties by lower index) -> renormalize; matches
    jax.nn.softmax + jax.lax.top_k semantics in float32."""
    logits = router_logits.astype(np.float32, copy=False)
    m = logits.max(axis=-1, keepdims=True)
    e = np.exp(logits - m)
    probs = e / e.sum(axis=-1, keepdims=True)
    top_idx = np.argsort(-probs, axis=-1, kind="stable")[:, :TOP_K]
    top_vals = np.take_along_axis(probs, top_idx, axis=-1)
    top_vals = top_vals / top_vals.sum(axis=-1, keepdims=True)
    return top_idx.astype(np.int64), top_vals.astype(np.float32)


def _np_cdt():
    return np.float16 if COMPUTE_DT == "fp16" else np.float32


def _pack_w1(w1e: np.ndarray) -> np.ndarray:
    # [I, H] -> [MT1, 128, KT1, 128] with [m, p, kt, j] = w1e[m*128+j, kt*128+p]
    return np.ascontiguousarray(
        w1e.reshape(MT1, 128, KT1, 128).transpose(0, 3, 2, 1).astype(_np_cdt())
    )


def _pack_w2(w2e: np.ndarray) -> np.ndarray:
    # [H, I] -> [MT2, 128, KT2, 128] with [m, p, kt, j] = w2e[m*128+j, kt*128+p]
    return np.ascontiguousarray(
        w2e.reshape(MT2, 128, KT2, 128).transpose(0, 3, 2, 1).astype(_np_cdt())
    )


def _prepare(x, router_logits, w1, w2):
    x = np.ascontiguousarray(np.asarray(x, dtype=np.float32))
    router_logits = np.asarray(router_logits, dtype=np.float32)
    w1 = np.asarray(w1, dtype=np.float32)
    w2 = np.asarray(w2, dtype=np.float32)
    T = x.shape[0]

    top_idx, top_gates = _route(router_logits)

    flat_e = top_idx.ravel()
    flat_t = np.repeat(np.arange(T), TOP_K)
    flat_g = top_gates.ravel()
    order = np.argsort(flat_e, kind="stable")
    st, sg = flat_t[order], flat_g[order]
    counts = np.bincount(flat_e, minlength=NUM_EXPERTS)
    starts = np.concatenate([[0], np.cumsum(counts)])
    toks = [st[starts[e] : starts[e + 1]] for e in range(NUM_EXPERTS)]
    gs = [sg[starts[e] : starts[e + 1]] for e in range(NUM_EXPERTS)]

    # pair the most-loaded expert with the least-loaded, 2 experts per core
    rank = np.argsort(-counts, kind="stable")
    big = rank[:N_CORES]
    small = rank[N_CORES:][::-1]  # big[i] pairs with small[i]

    C0 = _cap64(counts[big].max())
    C1 = _cap64(counts[small].max())
    nc = _get_program(C0, C1)

    in_maps = []
    for c in range(N_CORES):
        pair = (int(big[c]), int(small[c]))
        caps = (C0, C1)
        im = {}
        for s, e in enumerate(pair):
            n = int(counts[e])
            xg = np.zeros((HIDDEN, caps[s]), _np_cdt())
            xg[:, :n] = x[toks[e]].T
            g = np.zeros((caps[s],), np.float32)
            g[:n] = gs[e]
            im[f"xg{s}"] = xg
            im[f"g{s}"] = np.broadcast_to(
                g.astype(_np_cdt()), (128, caps[s])
            ).copy()
        im["w1p"] = np.stack([_pack_w1(w1[e]) for e in pair])
        im["w2p"] = np.stack([_pack_w2(w2[e]) for e in pair])
        in_maps.append(im)

    meta = dict(T=T, counts=counts, toks=toks, big=big, small=small,
                caps=(C0, C1))
    return nc, in_maps, meta


def _combine(results, meta):
    out = np.zeros((meta["T"], HIDDEN), np.float32)
    for c in range(N_CORES):
        for s, e in enumerate((int(meta["big"][c]), int(meta["small"][c]))):
            n = int(meta["counts"][e])
            y = results[c][f"y{s}"]  # [HIDDEN, Cs], already gate-scaled
            out[meta["toks"][e]] += y[:, :n].T.astype(np.float32)
    return out


def kernel(x, router_logits, w1, w2):
    nc, in_maps, meta = _prepare(x, router_logits, w1, w2)
    res = run_bass_kernel_spmd(nc, in_maps, core_ids=list(range(N_CORES)))
    kernel._last_results = res
    kernel._bench_state = {"in_maps": in_maps, "caps": meta["caps"]}
    return _combine(res.results, meta)

